# revision 29
# baseline (speedup 1.0000x reference)
"""Distributed attention kernel for 8 TRN2 NeuronCores.

Problem: x[2,2048,1024] -> qkv proj -> 16-head attention (softmax then /scale
quirk) -> out proj + bias.

Sharding: core c handles heads {2c, 2c+1} for BOTH batches (head-parallel).
QKV projection computed per-core against a 384-column slice of w_qkv, with
q/k produced directly in transposed [d, s] layout (lhsT = w slice, rhs = xT,
x transposed on the host). Attention is computed with transposed logits
tiles (lT = kT_chunk.T @ qT -> [k, q]) so the exp output feeds the score@v
matmul as the moving operand with no on-chip transposes of the attention
matrix; an appended ones-column in the v stationary operand yields softmax
sums for free, and score@v is software-pipelined one k-chunk behind qk so
the PE never waits on ScalarE's exp. Normalized attention outputs (already
in [inner, s] layout) are redistributed with four 8-core AllToAlls (one per
(batch, head), each fired as soon as its unit finishes so only the last
~1MB exchange is exposed); each core then projects one (batch, s/4) output
slice against the full (host-row-permuted) w_out.

All matmuls run as float32r (fp32 storage, reduced-precision PE mode, same
streaming rate as bf16 at N=512). Every matmul is padded to the full
128x128 array (zero-padded per-head q tiles, zero-padded v columns):
partial-array matmuls don't register as busy for the HAM clock gate, which
otherwise holds the PE at 1.2 GHz instead of 2.4 GHz. Do NOT mix bf16 and
f32r matmuls in this kernel - that combination produced nondeterministic
weight corruption on hardware. Batch-1 QKV work is emitted as fine-grained
filler interleaved into batch-0's attention loops to keep the PE dense.
"""

import numpy as np

S = 2048          # sequence length
D = 1024          # model dim
NH = 16           # total heads
DH = 64           # head dim
HPC = 2           # heads per core
NCORES = 8
KC = 8            # k-chunks of D (128 each)
QH = 2            # q halves (1024 each) per attention unit
SCALE_INV = 8.0   # 1 / (DH ** -0.5)

_CACHE = {}


def _ensure_paths():
    import sys
    for p in ("/opt/trn_rl_repo", "/root/.axon_site"):
        if p not in sys.path:
            sys.path.insert(0, p)


def _build_nc(debug_taps=False):
    _ensure_paths()
    from contextlib import ExitStack
    import concourse.bass as bass
    import concourse.mybir as mybir
    import concourse.tile as tile
    from concourse import bacc
    from concourse.masks import make_identity

    f32 = mybir.dt.float32
    f32r = mybir.dt.float32r
    bf16 = mybir.dt.bfloat16
    EXP = mybir.ActivationFunctionType.Exp

    nc = bacc.Bacc(None)
    xT_ext = nc.declare_dram_parameter("xT", [2, KC, 128, S], f32r, isOutput=False)
    wqkv_ext = nc.declare_dram_parameter("w_qkv", [KC, 128, 3 * HPC * DH], f32r, isOutput=False)
    wout_ext = nc.declare_dram_parameter("w_out", [KC, 128, D], f32r, isOutput=False)
    bout_ext = nc.declare_dram_parameter("b_out", [D], f32, isOutput=False)
    outA_ext = nc.declare_dram_parameter("outA", [512, D], f32, isOutput=True)
    outB_ext = nc.declare_dram_parameter("outB", [512, D], f32, isOutput=True)
    dbg = {}
    if debug_taps:
        dbg["qT"] = nc.declare_dram_parameter("dbg_qT", [128, S], f32, isOutput=True)
        dbg["kT"] = nc.declare_dram_parameter("dbg_kT", [128, S], f32, isOutput=True)
        dbg["outT"] = nc.declare_dram_parameter("dbg_outT", [DH + 1, 1024], f32, isOutput=True)
        dbg["recip"] = nc.declare_dram_parameter("dbg_recip", [1, 1024], f32, isOutput=True)
        dbg["bc"] = nc.declare_dram_parameter("dbg_bc", [DH, 1024], f32, isOutput=True)
        dbg["stage"] = nc.declare_dram_parameter("dbg_stage", [128, S], f32, isOutput=True)
        dbg["g"] = nc.declare_dram_parameter("dbg_g", [HPC, NCORES, DH, 512], f32, isOutput=True)

    with tile.TileContext(nc) as tc, ExitStack() as ctx:
        ctx.enter_context(
            nc.allow_low_precision(reason="float32r is fp32-width storage")
        )
        const = ctx.enter_context(tc.tile_pool(name="const", bufs=1))
        stage_pool = ctx.enter_context(tc.tile_pool(name="stg", bufs=2))
        recip_pool = ctx.enter_context(tc.tile_pool(name="rcp", bufs=1))
        bc_pool = ctx.enter_context(tc.tile_pool(name="bc", bufs=1))
        attn_ctx = ctx.enter_context(ExitStack())
        wq_pool = attn_ctx.enter_context(tc.tile_pool(name="wq", bufs=KC))
        qk_pool = attn_ctx.enter_context(tc.tile_pool(name="qk", bufs=6))
        vt_pool = attn_ctx.enter_context(tc.tile_pool(name="vt", bufs=1))
        vo_pool = attn_ctx.enter_context(tc.tile_pool(name="vo", bufs=18))
        st_pool = attn_ctx.enter_context(tc.tile_pool(name="st", bufs=2))

        ps_a = ctx.enter_context(tc.tile_pool(name="psA", bufs=2, space="PSUM"))
        ps_lt = ctx.enter_context(tc.tile_pool(name="psLT", bufs=2, space="PSUM"))
        ps_ot = ctx.enter_context(tc.tile_pool(name="psOT", bufs=1, space="PSUM"))
        dram = ctx.enter_context(tc.tile_pool(name="dram", bufs=1, space="DRAM"))

        a2a_in = dram.tile([HPC, NCORES, DH, 512], f32r, tag="a2a_in", name="a2a_in")
        a2a_out = dram.tile([2, HPC, NCORES, DH, 512], f32r, tag="a2a_out", name="a2a_out")

        # ---- constants / weights ----
        ident = const.tile([128, 128], f32, tag="ident", name="ident")
        make_identity(nc, ident)
        ones2 = const.tile([128, HPC, 1], f32, tag="ones2", name="ones2")
        nc.vector.memset(ones2, 1.0)
        zeros2 = const.tile([128, HPC, 128 - DH - 1], f32, tag="zeros2", name="zeros2")
        nc.vector.memset(zeros2, 0.0)
        zpad = const.tile([DH, 512], f32r, tag="zpad", name="zpad")
        zscr = const.tile([DH, 512], f32, tag="zscr", name="zscr")
        nc.vector.memset(zscr, 0.0)
        nc.vector.tensor_copy(zpad, zscr)
        bias_sb = const.tile([128, D], f32, tag="bias", name="bias_sb")
        bias_ap = bout_ext.ap()
        bias_bcast = bass.AP(
            tensor=bias_ap.tensor,
            offset=bias_ap.offset,
            ap=[[0, 128]] + [list(p) for p in bias_ap.ap],
        )
        nc.sync.dma_start(out=bias_sb, in_=bias_bcast)

        wq_sb = []
        for k in range(KC):
            t = wq_pool.tile([128, 3 * HPC * DH], f32r, tag="wq", name=f"wq{k}")
            nc.sync.dma_start(out=t, in_=wqkv_ext[k])
            wq_sb.append(t)

        qT = {}
        kT = {}
        vo = {}

        def qkv_work(b, xt_pool):
            # DMAs emitted now; compute returned as a step generator so it
            # can be drained as PE filler inside attention loops.
            xts = [None] * (4 * KC)
            # nkk-major order: the first compute chain consumes (k, nkk=0) for
            # all k, so those 8 transfers must lead the queue.
            for nkk in range(4):
                for k in range(KC):
                    t = xt_pool.tile([128, 512], f32r, tag="xt", name=f"xt{b}_{k}_{nkk}")
                    nc.sync.dma_start(
                        out=t, in_=xT_ext[b, k][:, nkk * 512:(nkk + 1) * 512]
                    )
                    xts[k * 4 + nkk] = t
            # q is zero-padded per head to full K=128 (a half-K matmul leaves
            # the PE array half-idle and never warms the HAM clock gate); k
            # stays compact because the other head's rows multiply against
            # q's zero rows and contribute nothing.
            qT[b] = [
                qk_pool.tile([128, S], f32r, tag="qk", name=f"qT{b}_{h}")
                for h in range(HPC)
            ]
            kT[b] = qk_pool.tile([128, S], f32r, tag="qk", name=f"kT{b}")
            vo[b] = [None] * (S // 128)

            def gen():
                for h in range(HPC):
                    r0 = 64 * (1 - h)
                    for c in range(4):
                        nc.vector.tensor_copy(
                            qT[b][h][r0:r0 + 64, c * 512:(c + 1) * 512], zpad
                        )
                yield
                vT = vt_pool.tile([HPC * DH, S], f32, tag="vt", name=f"vT{b}")
                for dst, c0 in ((vT, 256), (qT[b], 0), (kT[b], 128)):
                    for nkk in range(4):
                        ps = ps_a.tile([128, 512], f32, tag="psA", name=f"qkvps{b}_{c0}_{nkk}")
                        for k in range(KC):
                            nc.tensor.matmul(
                                ps,
                                lhsT=wq_sb[k][:, c0:c0 + 128],
                                rhs=xts[k * 4 + nkk],
                                start=(k == 0),
                                stop=(k == KC - 1),
                            )
                            yield
                        if isinstance(dst, list):
                            for h in range(HPC):
                                nc.vector.tensor_copy(
                                    dst[h][64 * h:64 * h + 64, nkk * 512:(nkk + 1) * 512],
                                    ps[64 * h:64 * h + 64, :],
                                )
                        else:
                            nc.vector.tensor_copy(dst[:, nkk * 512:(nkk + 1) * 512], ps)
                        yield
                # v to standard [s, d] layout via PE transpose; append ones col
                for sc in range(S // 128):
                    vps = ps_a.tile([128, 128], f32, tag="psA", name=f"vps{b}_{sc}")
                    nc.tensor.transpose(vps, vT[:, sc * 128:(sc + 1) * 128], ident)
                    vt = vo_pool.tile([128, HPC, 128], f32r, tag="vo", name=f"vo{b}_{sc}")
                    nc.vector.tensor_copy(
                        vt[:, :, 0:DH], vps.rearrange("p (h d) -> p h d", h=HPC)
                    )
                    nc.vector.tensor_copy(vt[:, :, DH:DH + 1], ones2)
                    nc.vector.tensor_copy(vt[:, :, DH + 1:], zeros2)
                    vo[b][sc] = vt
                    yield

            return gen()

        def drain(it, n=None):
            if n is None:
                for _ in it:
                    pass
            else:
                for _ in range(n):
                    if next(it, StopIteration) is StopIteration:
                        break

        def attention_unit(b, hh, stage, filler=None, rate=0):
            h0 = hh * DH
            NK = S // 128
            for qh in range(QH):
                q0 = qh * 1024

                def sv(k, st):
                    for half in range(2):
                        nc.tensor.matmul(
                            outT[:, half * 512:(half + 1) * 512],
                            lhsT=vo[b][k][:, hh, :],
                            rhs=st[:, half * 512:(half + 1) * 512],
                            start=(k == 0),
                            stop=(k == NK - 1),
                        )

                outT = ps_ot.tile([128, 1024], f32, tag="psOT", name=f"outT{b}_{hh}_{qh}")
                pending = None
                for k in range(NK):
                    lt = ps_lt.tile([128, 1024], f32, tag="psLT", name=f"lt{b}_{hh}_{qh}_{k}")
                    for half in range(2):
                        nc.tensor.matmul(
                            lt[:, half * 512:(half + 1) * 512],
                            lhsT=kT[b][:, k * 128:(k + 1) * 128],
                            rhs=qT[b][hh][:, q0 + half * 512:q0 + (half + 1) * 512],
                            start=True,
                            stop=True,
                        )
                    st = st_pool.tile([128, 1024], f32r, tag="st", name=f"st{b}_{hh}_{qh}_{k}")
                    nc.scalar.activation(st, lt, EXP)
                    if pending is not None:
                        sv(*pending)
                    pending = (k, st)
                    if filler is not None:
                        drain(filler, rate)
                sv(*pending)
                # normalize: stage[d, q] = outT[d, q] * (8 / sums[q])
                if debug_taps and b == 0 and hh == 0 and qh == 0:
                    ot_sb = bc_pool.tile([DH + 1, 1024], f32, tag="bc", name="dbg_ot_sb")
                    nc.vector.tensor_copy(ot_sb, outT)
                    nc.sync.dma_start(out=dbg["outT"].ap(), in_=ot_sb)
                recip = recip_pool.tile([1, 1024], f32, tag="rcp", name=f"rcp{b}_{hh}_{qh}")
                nc.vector.reciprocal(recip, outT[DH:DH + 1, :])
                if debug_taps and b == 0 and hh == 0 and qh == 0:
                    nc.sync.dma_start(out=dbg["recip"].ap(), in_=recip)
                bc_sb = bc_pool.tile([DH, 1024], f32, tag="bc", name=f"bc{b}_{hh}_{qh}")
                nc.gpsimd.partition_broadcast(bc_sb, recip)
                nc.vector.scalar_tensor_tensor(
                    out=stage[h0:h0 + DH, q0:q0 + 1024],
                    in0=outT[0:DH, :],
                    scalar=SCALE_INV,
                    in1=bc_sb,
                    op0=mybir.AluOpType.mult,
                    op1=mybir.AluOpType.mult,
                )
                if debug_taps and b == 0 and hh == 0 and qh == 0:
                    nc.sync.dma_start(out=dbg["bc"].ap(), in_=bc_sb)

        def finish_batch(b, filler=None, rate=0):
            stage = stage_pool.tile([HPC * DH, S], f32r, tag="stg", name=f"stg{b}")
            for hh in range(HPC):
                attention_unit(b, hh, stage, filler, rate)
                # ship this unit's rows into its head-call slots and fire the
                # exchange: it overlaps the next unit / batch compute, and
                # cores j with B(j)==b receive all slots they need.
                h0 = hh * DH
                for qq in range(4):
                    nc.gpsimd.dma_start(
                        out=a2a_in[hh, 4 * b + qq],
                        in_=stage[h0:h0 + DH, qq * 512:(qq + 1) * 512],
                    )
                nc.gpsimd.collective_compute(
                    "AllToAll",
                    mybir.AluOpType.bypass,
                    replica_groups=[list(range(NCORES))],
                    ins=[a2a_in[hh].opt()],
                    outs=[a2a_out[b, hh].opt()],
                )
            if debug_taps and b == 0:
                nc.sync.dma_start(out=dbg["stage"].ap(), in_=stage[:].bitcast(f32))
                nc.sync.dma_start(out=dbg["qT"].ap(), in_=qT[0][0][:].bitcast(f32))
                nc.sync.dma_start(out=dbg["kT"].ap(), in_=kT[0][:].bitcast(f32))

        def proj_work(half, oext, wo_sb, g_pool, y_pool):
            g_sb = []
            for k in range(KC):
                hh, cc = (0, k) if k < 4 else (1, k - 4)
                t = g_pool.tile([128, 512], f32r, tag="g", name=f"g{half}_{k}")
                nc.sync.dma_start(
                    out=t,
                    in_=a2a_out[half, hh, 2 * cc:2 * cc + 2].rearrange(
                        "s d c -> (s d) c"
                    ),
                )
                g_sb.append(t)

            def gen():
                for sc in range(4):
                    y_sb = y_pool.tile([128, D], f32, tag="y", name=f"y{half}_{sc}")
                    for nk in range(2):
                        yps = ps_a.tile([128, 512], f32, tag="psA", name=f"yps{half}_{sc}_{nk}")
                        for k in range(KC):
                            nc.tensor.matmul(
                                yps,
                                lhsT=g_sb[k][:, sc * 128:(sc + 1) * 128],
                                rhs=wo_sb[k][:, nk * 512:(nk + 1) * 512],
                                start=(k == 0),
                                stop=(k == KC - 1),
                            )
                            yield
                        nc.vector.tensor_add(
                            y_sb[:, nk * 512:(nk + 1) * 512],
                            yps,
                            bias_sb[:, nk * 512:(nk + 1) * 512],
                        )
                        yield
                    nc.sync.dma_start(out=oext[sc * 128:(sc + 1) * 128, :], in_=y_sb)

            return gen()

        with tc.tile_pool(name="xt", bufs=34) as xt_pool:
            drain(qkv_work(0, xt_pool))
            g1 = qkv_work(1, xt_pool)
            finish_batch(0, filler=g1, rate=5)
            drain(g1)
            finish_batch(1)
        attn_ctx.close()
        # attention pools freed; open late pools for the projection passes
        wo_pool = ctx.enter_context(tc.tile_pool(name="wo", bufs=KC))
        g_pool = ctx.enter_context(tc.tile_pool(name="g", bufs=KC))
        y_pool = ctx.enter_context(tc.tile_pool(name="y", bufs=2))
        wo_sb = []
        for k in range(KC):
            t = wo_pool.tile([128, D], f32r, tag="wo", name=f"wo{k}")
            nc.sync.dma_start(out=t, in_=wout_ext[k])
            wo_sb.append(t)
        if debug_taps:
            nc.sync.dma_start(out=dbg["g"].ap(), in_=a2a_out[0].bitcast(f32))
        drain(proj_work(0, outA_ext, wo_sb, g_pool, y_pool))
        drain(proj_work(1, outB_ext, wo_sb, g_pool, y_pool))

    nc.finalize()
    return nc


def _prep_in_maps(x, w_qkv, w_out, b_out):
    x = np.ascontiguousarray(x, dtype=np.float32)
    w_qkv = np.ascontiguousarray(w_qkv, dtype=np.float32)
    w_out = np.ascontiguousarray(w_out, dtype=np.float32)
    b_out = np.ascontiguousarray(b_out, dtype=np.float32)

    xT = np.ascontiguousarray(
        np.stack([x[0].T, x[1].T]).reshape(2, KC, 128, S)
    )
    # arrival order after the per-(batch, head) AllToAll: call h delivers
    # head (2c+h) for c=0..7; stacked [call0 (512 rows), call1 (512 rows)].
    perm = []
    for h in range(HPC):
        for c in range(NCORES):
            base = 128 * c + DH * h
            perm.extend(range(base, base + DH))
    wo = np.ascontiguousarray(w_out[np.array(perm)].reshape(KC, 128, D))
    in_maps = []
    for c in range(NCORES):
        c0 = c * HPC * DH
        shard = np.concatenate(
            [
                w_qkv[:, c0:c0 + 128],
                w_qkv[:, D + c0:D + c0 + 128],
                w_qkv[:, 2 * D + c0:2 * D + c0 + 128],
            ],
            axis=1,
        )
        in_maps.append(
            {
                "xT": xT,
                "w_qkv": np.ascontiguousarray(shard.reshape(KC, 128, 3 * HPC * DH)),
                "w_out": wo,
                "b_out": b_out,
            }
        )
    return in_maps


def _run(x, w_qkv, w_out, b_out, trace=False, debug_taps=False):
    _ensure_paths()
    from concourse.bass_utils import run_bass_kernel_spmd

    key = ("nc_dbg" if debug_taps else "nc")
    if key not in _CACHE:
        _CACHE[key] = _build_nc(debug_taps=debug_taps)
    nc = _CACHE[key]
    in_maps = _prep_in_maps(x, w_qkv, w_out, b_out)
    res = run_bass_kernel_spmd(nc, in_maps, list(range(NCORES)), trace=trace)
    out = np.empty((2, S, D), dtype=np.float32)
    for c in range(NCORES):
        b, q = c // 4, c % 4
        key = "outA" if b == 0 else "outB"
        out[b, 512 * q:512 * (q + 1), :] = res.results[c][key]
    return out, res


def kernel(x, w_qkv, w_out, b_out):
    out, _ = _run(x, w_qkv, w_out, b_out, trace=False)
    return out


# revision 31
# speedup vs baseline: 1.1814x; 1.1814x over previous
"""Distributed attention kernel for 8 TRN2 NeuronCores.

Problem: x[2,2048,1024] -> qkv proj -> 16-head attention (softmax then /scale
quirk) -> out proj + bias.

Sharding: core c handles heads {2c, 2c+1} for BOTH batches (head-parallel).
QKV projection computed per-core against a 384-column slice of w_qkv, with
q/k produced directly in transposed [d, s] layout (lhsT = w slice, rhs = xT,
x transposed on the host). Attention is computed with transposed logits
tiles (lT = kT_chunk.T @ qT -> [k, q]) so the exp output feeds the score@v
matmul as the moving operand with no on-chip transposes of the attention
matrix; an appended ones-column in the v stationary operand yields softmax
sums for free, and score@v is software-pipelined one k-chunk behind qk so
the PE never waits on ScalarE's exp. Normalized attention outputs (already
in [inner, s] layout) are redistributed with four 8-core AllToAlls (one per
(batch, head), each fired as soon as its unit finishes so only the last
~1MB exchange is exposed); each core then projects one (batch, s/4) output
slice against the full (host-row-permuted) w_out.

All matmuls run as float32r (fp32 storage, reduced-precision PE mode, same
streaming rate as bf16 at N=512). Every matmul is padded to the full
128x128 array (zero-padded per-head q tiles, zero-padded v columns):
partial-array matmuls don't register as busy for the HAM clock gate, which
otherwise holds the PE at 1.2 GHz instead of 2.4 GHz. Do NOT mix bf16 and
f32r matmuls in this kernel - that combination produced nondeterministic
weight corruption on hardware. Batch-1 QKV work is emitted as fine-grained
filler interleaved into batch-0's attention loops to keep the PE dense.
"""

import numpy as np

S = 2048          # sequence length
D = 1024          # model dim
NH = 16           # total heads
DH = 64           # head dim
HPC = 2           # heads per core
NCORES = 8
KC = 8            # k-chunks of D (128 each)
QH = 2            # q halves (1024 each) per attention unit
SCALE_INV = 8.0   # 1 / (DH ** -0.5)

_CACHE = {}


def _ensure_paths():
    import sys
    for p in ("/opt/trn_rl_repo", "/root/.axon_site"):
        if p not in sys.path:
            sys.path.insert(0, p)


def _build_nc(debug_taps=False):
    _ensure_paths()
    from contextlib import ExitStack
    import concourse.bass as bass
    import concourse.mybir as mybir
    import concourse.tile as tile
    from concourse import bacc
    from concourse.masks import make_identity

    f32 = mybir.dt.float32
    f32r = mybir.dt.float32r
    bf16 = mybir.dt.bfloat16
    EXP = mybir.ActivationFunctionType.Exp

    nc = bacc.Bacc(None)
    xT_ext = nc.declare_dram_parameter("xT", [2, KC, 128, S], f32r, isOutput=False)
    wqkv_ext = nc.declare_dram_parameter("w_qkv", [KC, 128, 3 * HPC * DH], f32r, isOutput=False)
    wout_ext = nc.declare_dram_parameter("w_out", [KC, 128, D], f32r, isOutput=False)
    bout_ext = nc.declare_dram_parameter("b_out", [D], f32, isOutput=False)
    outA_ext = nc.declare_dram_parameter("outA", [512, D], f32, isOutput=True)
    outB_ext = nc.declare_dram_parameter("outB", [512, D], f32, isOutput=True)
    dbg = {}
    if debug_taps:
        dbg["qT"] = nc.declare_dram_parameter("dbg_qT", [128, S], f32, isOutput=True)
        dbg["kT"] = nc.declare_dram_parameter("dbg_kT", [128, S], f32, isOutput=True)
        dbg["outT"] = nc.declare_dram_parameter("dbg_outT", [DH + 1, 1024], f32, isOutput=True)
        dbg["recip"] = nc.declare_dram_parameter("dbg_recip", [1, 1024], f32, isOutput=True)
        dbg["bc"] = nc.declare_dram_parameter("dbg_bc", [DH, 1024], f32, isOutput=True)
        dbg["stage"] = nc.declare_dram_parameter("dbg_stage", [128, S], f32, isOutput=True)
        dbg["g"] = nc.declare_dram_parameter("dbg_g", [HPC, NCORES, DH, 512], f32, isOutput=True)

    with tile.TileContext(nc) as tc, ExitStack() as ctx:
        ctx.enter_context(
            nc.allow_low_precision(reason="float32r is fp32-width storage")
        )
        const = ctx.enter_context(tc.tile_pool(name="const", bufs=1))
        stage_pool = ctx.enter_context(tc.tile_pool(name="stg", bufs=2))
        recip_pool = ctx.enter_context(tc.tile_pool(name="rcp", bufs=1))
        bc_pool = ctx.enter_context(tc.tile_pool(name="bc", bufs=1))
        ot_pool = ctx.enter_context(tc.tile_pool(name="ot", bufs=1))
        attn_ctx = ctx.enter_context(ExitStack())
        wq_pool = attn_ctx.enter_context(tc.tile_pool(name="wq", bufs=KC))
        qk_pool = attn_ctx.enter_context(tc.tile_pool(name="qk", bufs=6))
        vt_pool = attn_ctx.enter_context(tc.tile_pool(name="vt", bufs=1))
        vo_pool = attn_ctx.enter_context(tc.tile_pool(name="vo", bufs=16))
        st_pool = attn_ctx.enter_context(tc.tile_pool(name="st", bufs=2))

        ps_a = ctx.enter_context(tc.tile_pool(name="psA", bufs=2, space="PSUM"))
        ps_lt = ctx.enter_context(tc.tile_pool(name="psLT", bufs=2, space="PSUM"))
        ps_ot = ctx.enter_context(tc.tile_pool(name="psOT", bufs=1, space="PSUM"))
        dram = ctx.enter_context(tc.tile_pool(name="dram", bufs=1, space="DRAM"))

        a2a_in = dram.tile([HPC, NCORES, DH, 512], f32r, tag="a2a_in", name="a2a_in")
        a2a_out = dram.tile([2, HPC, NCORES, DH, 512], f32r, tag="a2a_out", name="a2a_out")

        # ---- constants / weights ----
        ident = const.tile([128, 128], f32, tag="ident", name="ident")
        make_identity(nc, ident)
        ones2 = const.tile([128, HPC, 1], f32, tag="ones2", name="ones2")
        nc.vector.memset(ones2, 1.0)
        zeros2 = const.tile([128, HPC, 128 - DH - 1], f32, tag="zeros2", name="zeros2")
        nc.vector.memset(zeros2, 0.0)
        zpad = const.tile([DH, 512], f32r, tag="zpad", name="zpad")
        zscr = const.tile([DH, 512], f32, tag="zscr", name="zscr")
        nc.vector.memset(zscr, 0.0)
        nc.vector.tensor_copy(zpad, zscr)
        bias_sb = const.tile([128, D], f32, tag="bias", name="bias_sb")

        def load_bias():
            bias_ap = bout_ext.ap()
            bias_bcast = bass.AP(
                tensor=bias_ap.tensor,
                offset=bias_ap.offset,
                ap=[[0, 128]] + [list(p) for p in bias_ap.ap],
            )
            nc.sync.dma_start(out=bias_sb, in_=bias_bcast)

        wq_sb = []
        for k in range(KC):
            t = wq_pool.tile([128, 3 * HPC * DH], f32r, tag="wq", name=f"wq{k}")
            nc.sync.dma_start(out=t, in_=wqkv_ext[k])
            wq_sb.append(t)

        qT = {}
        kT = {}
        vo = {}

        def qkv_work(b, xt_pool):
            # DMAs emitted now; compute returned as a step generator so it
            # can be drained as PE filler inside attention loops.
            xts = [None] * (4 * KC)
            # nkk-major order: the first compute chain consumes (k, nkk=0) for
            # all k, so those 8 transfers must lead the queue.
            for nkk in range(4):
                for k in range(KC):
                    t = xt_pool.tile([128, 512], f32r, tag="xt", name=f"xt{b}_{k}_{nkk}")
                    nc.sync.dma_start(
                        out=t, in_=xT_ext[b, k][:, nkk * 512:(nkk + 1) * 512]
                    )
                    xts[k * 4 + nkk] = t
            # q is zero-padded per head to full K=128 (a half-K matmul leaves
            # the PE array half-idle and never warms the HAM clock gate); k
            # stays compact because the other head's rows multiply against
            # q's zero rows and contribute nothing.
            qT[b] = [
                qk_pool.tile([128, S], f32r, tag="qk", name=f"qT{b}_{h}")
                for h in range(HPC)
            ]
            kT[b] = qk_pool.tile([128, S], f32r, tag="qk", name=f"kT{b}")
            vo[b] = [None] * (S // 128)

            def gen():
                for h in range(HPC):
                    r0 = 64 * (1 - h)
                    for c in range(4):
                        nc.vector.tensor_copy(
                            qT[b][h][r0:r0 + 64, c * 512:(c + 1) * 512], zpad
                        )
                yield
                vT = vt_pool.tile([HPC * DH, S], f32, tag="vt", name=f"vT{b}")
                for dst, c0 in ((vT, 256), (qT[b], 0), (kT[b], 128)):
                    for nkk in range(4):
                        ps = ps_a.tile([128, 512], f32, tag="psA", name=f"qkvps{b}_{c0}_{nkk}")
                        for k in range(KC):
                            nc.tensor.matmul(
                                ps,
                                lhsT=wq_sb[k][:, c0:c0 + 128],
                                rhs=xts[k * 4 + nkk],
                                start=(k == 0),
                                stop=(k == KC - 1),
                            )
                            yield
                        if isinstance(dst, list):
                            for h in range(HPC):
                                nc.vector.tensor_copy(
                                    dst[h][64 * h:64 * h + 64, nkk * 512:(nkk + 1) * 512],
                                    ps[64 * h:64 * h + 64, :],
                                )
                        else:
                            nc.vector.tensor_copy(dst[:, nkk * 512:(nkk + 1) * 512], ps)
                        yield
                # v to standard [s, d] layout via PE transpose; append ones col
                for sc in range(S // 128):
                    vps = ps_a.tile([128, 128], f32, tag="psA", name=f"vps{b}_{sc}")
                    nc.tensor.transpose(vps, vT[:, sc * 128:(sc + 1) * 128], ident)
                    vt = vo_pool.tile([128, HPC, 128], f32r, tag="vo", name=f"vo{b}_{sc}")
                    nc.vector.tensor_copy(
                        vt[:, :, 0:DH], vps.rearrange("p (h d) -> p h d", h=HPC)
                    )
                    nc.vector.tensor_copy(vt[:, :, DH:DH + 1], ones2)
                    nc.vector.tensor_copy(vt[:, :, DH + 1:], zeros2)
                    vo[b][sc] = vt
                    yield

            return gen()

        def drain(it, n=None):
            if n is None:
                for _ in it:
                    pass
            else:
                for _ in range(n):
                    if next(it, StopIteration) is StopIteration:
                        break

        def attention_unit(b, hh, stage, filler=None, rate=0):
            h0 = hh * DH
            NK = S // 128
            for qh in range(QH):
                q0 = qh * 1024

                def sv(k, st):
                    for half in range(2):
                        nc.tensor.matmul(
                            outT[:, half * 512:(half + 1) * 512],
                            lhsT=vo[b][k][:, hh, :],
                            rhs=st[:, half * 512:(half + 1) * 512],
                            start=(k == 0),
                            stop=(k == NK - 1),
                        )

                outT = ps_ot.tile([128, 1024], f32, tag="psOT", name=f"outT{b}_{hh}_{qh}")
                pending = None
                for k in range(NK):
                    lt = ps_lt.tile([128, 1024], f32, tag="psLT", name=f"lt{b}_{hh}_{qh}_{k}")
                    for half in range(2):
                        nc.tensor.matmul(
                            lt[:, half * 512:(half + 1) * 512],
                            lhsT=kT[b][:, k * 128:(k + 1) * 128],
                            rhs=qT[b][hh][:, q0 + half * 512:q0 + (half + 1) * 512],
                            start=True,
                            stop=True,
                        )
                    st = st_pool.tile([128, 1024], f32r, tag="st", name=f"st{b}_{hh}_{qh}_{k}")
                    nc.scalar.activation(st, lt, EXP)
                    if pending is not None:
                        sv(*pending)
                    pending = (k, st)
                    if filler is not None:
                        drain(filler, rate)
                sv(*pending)
                # evacuate outT fast (ScalarE copy) so the PSUM slot frees in
                # ~1.1us instead of being held through the ~9us normalize
                # (the single-lane reciprocal was stalling the next qh's
                # accumulation start by ~7us).
                ot_sb = ot_pool.tile([DH + 1, 1024], f32, tag="ot", name=f"ot{b}_{hh}_{qh}")
                nc.scalar.copy(ot_sb, outT[0:DH + 1, :])
                if debug_taps and b == 0 and hh == 0 and qh == 0:
                    nc.sync.dma_start(out=dbg["outT"].ap(), in_=ot_sb)
                recip = recip_pool.tile([1, 1024], f32, tag="rcp", name=f"rcp{b}_{hh}_{qh}")
                nc.vector.reciprocal(recip, ot_sb[DH:DH + 1, :])
                if debug_taps and b == 0 and hh == 0 and qh == 0:
                    nc.sync.dma_start(out=dbg["recip"].ap(), in_=recip)
                bc_sb = bc_pool.tile([DH, 1024], f32, tag="bc", name=f"bc{b}_{hh}_{qh}")
                nc.gpsimd.partition_broadcast(bc_sb, recip)
                nc.vector.scalar_tensor_tensor(
                    out=stage[h0:h0 + DH, q0:q0 + 1024],
                    in0=ot_sb[0:DH, :],
                    scalar=SCALE_INV,
                    in1=bc_sb,
                    op0=mybir.AluOpType.mult,
                    op1=mybir.AluOpType.mult,
                )
                if debug_taps and b == 0 and hh == 0 and qh == 0:
                    nc.sync.dma_start(out=dbg["bc"].ap(), in_=bc_sb)

        def finish_batch(b, filler=None, rate=0):
            stage = stage_pool.tile([HPC * DH, S], f32r, tag="stg", name=f"stg{b}")
            for hh in range(HPC):
                attention_unit(b, hh, stage, filler, rate)
                # ship this unit's rows into its head-call slots and fire the
                # exchange: it overlaps the next unit / batch compute, and
                # cores j with B(j)==b receive all slots they need.
                h0 = hh * DH
                for qq in range(4):
                    nc.gpsimd.dma_start(
                        out=a2a_in[hh, 4 * b + qq],
                        in_=stage[h0:h0 + DH, qq * 512:(qq + 1) * 512],
                    )
                nc.gpsimd.collective_compute(
                    "AllToAll",
                    mybir.AluOpType.bypass,
                    replica_groups=[list(range(NCORES))],
                    ins=[a2a_in[hh].opt()],
                    outs=[a2a_out[b, hh].opt()],
                )
            if debug_taps and b == 0:
                nc.sync.dma_start(out=dbg["stage"].ap(), in_=stage[:].bitcast(f32))
                nc.sync.dma_start(out=dbg["qT"].ap(), in_=qT[0][0][:].bitcast(f32))
                nc.sync.dma_start(out=dbg["kT"].ap(), in_=kT[0][:].bitcast(f32))

        def proj_work(half, oext, wo_sb, g_pool, y_pool):
            g_sb = []
            for k in range(KC):
                hh, cc = (0, k) if k < 4 else (1, k - 4)
                t = g_pool.tile([128, 512], f32r, tag="g", name=f"g{half}_{k}")
                nc.sync.dma_start(
                    out=t,
                    in_=a2a_out[half, hh, 2 * cc:2 * cc + 2].rearrange(
                        "s d c -> (s d) c"
                    ),
                )
                g_sb.append(t)

            def gen():
                for sc in range(4):
                    y_sb = y_pool.tile([128, D], f32, tag="y", name=f"y{half}_{sc}")
                    for nk in range(2):
                        yps = ps_a.tile([128, 512], f32, tag="psA", name=f"yps{half}_{sc}_{nk}")
                        for k in range(KC):
                            nc.tensor.matmul(
                                yps,
                                lhsT=g_sb[k][:, sc * 128:(sc + 1) * 128],
                                rhs=wo_sb[k][:, nk * 512:(nk + 1) * 512],
                                start=(k == 0),
                                stop=(k == KC - 1),
                            )
                            yield
                        nc.vector.tensor_add(
                            y_sb[:, nk * 512:(nk + 1) * 512],
                            yps,
                            bias_sb[:, nk * 512:(nk + 1) * 512],
                        )
                        yield
                    nc.sync.dma_start(out=oext[sc * 128:(sc + 1) * 128, :], in_=y_sb)

            return gen()

        with tc.tile_pool(name="xt", bufs=33) as xt_pool:
            drain(qkv_work(0, xt_pool))
            g1 = qkv_work(1, xt_pool)
            finish_batch(0, filler=g1, rate=5)
            drain(g1)
        # xt freed: the projection-A pools fit alongside the attention pools,
        # so projection A (deps ready mid-batch-1) weaves into b1's PE gaps.
        wo_pool = ctx.enter_context(tc.tile_pool(name="wo", bufs=KC))
        g_pool = ctx.enter_context(tc.tile_pool(name="g", bufs=KC))
        y_pool = ctx.enter_context(tc.tile_pool(name="y", bufs=2))
        load_bias()
        wo_sb = []
        for k in range(KC):
            t = wo_pool.tile([128, D], f32r, tag="wo", name=f"wo{k}")
            nc.sync.dma_start(out=t, in_=wout_ext[k])
            wo_sb.append(t)
        if debug_taps:
            nc.sync.dma_start(out=dbg["g"].ap(), in_=a2a_out[0].bitcast(f32))
        drain(proj_work(0, outA_ext, wo_sb, g_pool, y_pool))
        finish_batch(1)
        drain(proj_work(1, outB_ext, wo_sb, g_pool, y_pool))

    nc.finalize()
    return nc


def _prep_in_maps(x, w_qkv, w_out, b_out):
    x = np.ascontiguousarray(x, dtype=np.float32)
    w_qkv = np.ascontiguousarray(w_qkv, dtype=np.float32)
    w_out = np.ascontiguousarray(w_out, dtype=np.float32)
    b_out = np.ascontiguousarray(b_out, dtype=np.float32)

    xT = np.ascontiguousarray(
        np.stack([x[0].T, x[1].T]).reshape(2, KC, 128, S)
    )
    # arrival order after the per-(batch, head) AllToAll: call h delivers
    # head (2c+h) for c=0..7; stacked [call0 (512 rows), call1 (512 rows)].
    perm = []
    for h in range(HPC):
        for c in range(NCORES):
            base = 128 * c + DH * h
            perm.extend(range(base, base + DH))
    wo = np.ascontiguousarray(w_out[np.array(perm)].reshape(KC, 128, D))
    in_maps = []
    for c in range(NCORES):
        c0 = c * HPC * DH
        shard = np.concatenate(
            [
                w_qkv[:, c0:c0 + 128],
                w_qkv[:, D + c0:D + c0 + 128],
                w_qkv[:, 2 * D + c0:2 * D + c0 + 128],
            ],
            axis=1,
        )
        in_maps.append(
            {
                "xT": xT,
                "w_qkv": np.ascontiguousarray(shard.reshape(KC, 128, 3 * HPC * DH)),
                "w_out": wo,
                "b_out": b_out,
            }
        )
    return in_maps


def _run(x, w_qkv, w_out, b_out, trace=False, debug_taps=False):
    _ensure_paths()
    from concourse.bass_utils import run_bass_kernel_spmd

    key = ("nc_dbg" if debug_taps else "nc")
    if key not in _CACHE:
        _CACHE[key] = _build_nc(debug_taps=debug_taps)
    nc = _CACHE[key]
    in_maps = _prep_in_maps(x, w_qkv, w_out, b_out)
    res = run_bass_kernel_spmd(nc, in_maps, list(range(NCORES)), trace=trace)
    out = np.empty((2, S, D), dtype=np.float32)
    for c in range(NCORES):
        b, q = c // 4, c % 4
        key = "outA" if b == 0 else "outB"
        out[b, 512 * q:512 * (q + 1), :] = res.results[c][key]
    return out, res


def kernel(x, w_qkv, w_out, b_out):
    out, _ = _run(x, w_qkv, w_out, b_out, trace=False)
    return out
